# revision 4
# baseline (speedup 1.0000x reference)
"""CAREConv Trainium2 kernel — 8-core SPMD Bass implementation (v6).

Pipeline (per core; nodes in-degree-sorted, dealt to cores per round by
out-degree quota):
  A. t = tanh(feat @ W_mlp + b) in f32 for the core's node shard (PE
     matmul + ACT tanh; selection is exquisitely sensitive to distance
     perturbations, so the whole distance path stays f32), written into
     the t-columns of the core's G_loc rows (feat-bf16 columns
     host-staged); AllGather -> full G table [NTOT, 512B rows:
     t f32 40 | pad | feat bf16 128] in DRAM. A small core-local gather
     (overlapping the AllGather) recovers t_own/own-feat in dst order —
     the G-table row order is decoupled from dst tiling so low-out-degree
     nodes can occupy single-window-covered row regions (less ELL pad).
  B. Per 128-node tile (grouped): ONE bulk-gather G[src] per edge slot
     (512B elements, int16 window-local indices, three overlapping
     address windows), L1 edge distance from the t part (DVE subtract +
     abs-sum reduce), per-node exact top-k threshold via iterated
     max8/match_replace + tie-aware prefix scan; then ACT scales each
     slot's bf16 feat by its mask/k (per-partition scalar) and the PE
     accumulates the transposed messages directly into group PSUM via
     identity-rhs matmuls (f32 accumulation), finishing with the fused
     (feat + hr) @ W_lin + b matmul.
Host does integer/topology preprocessing only (ELL layout, degrees,
k = ceil(p*deg), index lists, permutations, dtype staging) and output
unpermutation.
"""

import math
import os

import numpy as np

N_NODES = 50000
N_EDGES = 800000
D = 128
C = 40
GW = 128  # G row width in f32 (512B)
FOFF = 64  # f32 col offset of the bf16 feat block in a G row
P_KEEP = 0.5
NCORES = 8
TP = 128
NEG_BIG = -1.0e30
WIN = 32768  # int16 index window
SBUDGET = int(os.environ.get("K_SBUDGET", "72"))  # max gather cols per group
GMAX = 8  # max tiles per group
GCH = int(os.environ.get("K_GCH", "8"))  # gather cols per call (idxs = GCH*128)
SCRATCH = int(os.environ.get("K_SCRATCH", str(48 * 1024)))  # SWDGE ring bytes


def _wrap_idx(idx_ell):
    """[128, ncols] window-local indices -> wrapped [128, 8*ncols] int16.

    Gather-list position j = col*128 + p; wrapped[q, col*8 + r] =
    idx_ell[16*r + q, col]; replicated across the 8 16-partition groups.
    """
    ncols = idx_ell.shape[1]
    w = np.zeros((16, 8 * ncols), np.int16)
    for r in range(8):
        w[:, r::8] = idx_ell[16 * r : 16 * r + 16, :]
    return np.tile(w, (8, 1))


def _preprocess(src, dst, feat, n_nodes, ncores):
    import ml_dtypes

    E = src.shape[0]
    deg = np.bincount(dst, minlength=n_nodes).astype(np.int64)
    outdeg = np.bincount(src, minlength=n_nodes).astype(np.int64)
    kk = (deg + 1) // 2

    perm = np.argsort(-deg, kind="stable")
    n_tiles = math.ceil(n_nodes / (TP * ncores)) * ncores
    NTOT = n_tiles * TP
    NPC = NTOT // ncores
    NSLOT = n_tiles // ncores

    # dst assignment: per in-degree round (ncores tiles of similar in-deg),
    # deal nodes to cores by OUT-degree quota: cores whose G-table row
    # blocks sit in single-window-covered regions (0,7, then 1,6) get the
    # lowest-out-degree nodes, which minimizes forced-window edge counts
    # and hence ELL padding.
    quota_order = [0, 7, 1, 6, 2, 5, 3, 4]
    fperm = np.full(NTOT, -1, np.int64)
    for r in range(NSLOT):
        ids = perm[r * TP * ncores : min((r + 1) * TP * ncores, n_nodes)]
        ids_s = ids[np.argsort(outdeg[ids], kind="stable")]
        for qi, c in enumerate(quota_order):
            chunk = ids_s[qi * TP : (qi + 1) * TP]
            base = c * NPC + r * TP
            fperm[base : base + len(chunk)] = chunk
    valid = fperm >= 0
    inv_f = np.full(n_nodes, -1, np.int64)
    inv_f[fperm[valid]] = np.nonzero(valid)[0]

    degf = np.zeros(NTOT, np.int64)
    degf[valid] = deg[fperm[valid]]
    kf = np.zeros(NTOT, np.int64)
    kf[valid] = kk[fperm[valid]]

    # edges sorted by final dst vid (stable); within each segment window-A
    # edges first, then window-B, original order within each part (slot
    # order only affects exact-duplicate ties, which are
    # src-interchangeable). Windows A=[0,WIN) and B=[NTOT-WIN,NTOT)
    # overlap in [NTOT-WIN, WIN); edges whose src falls in the overlap are
    # assigned to whichever window balances cA vs cB per dst node, which
    # minimizes ELL padding (SA+SB per tile).
    dstf = inv_f[dst]
    # three overlapping int16 windows: A=[0,WIN), M=[MOFF,MOFF+WIN),
    # B=[NTOT-WIN,NTOT), MOFF=(NTOT-WIN)//2. Edge categories by which
    # windows cover src: 1=A, 2=AM, 3=AMB, 4=MB, 5=B.
    MOFF = (NTOT - WIN) // 2
    BLO = NTOT - WIN

    # G-table row placement (grow) is decoupled from dst position: within
    # each core's row block, rows in regions covered by fewer windows get
    # the core's lowest-out-degree nodes (fewest forced-window edges).
    # t_own / own-feat are recovered on device by a small core-local gather
    # (srctO) from the core's own staging rows.
    prio = np.ones(NTOT, np.int8)
    prio[:MOFF] = 0
    prio[MOFF + WIN :] = 0
    prio[BLO:WIN] = 2
    growOf = np.empty(NTOT, np.int64)  # dst position -> table row
    od_pos = np.zeros(NTOT, np.int64)
    od_pos[valid] = outdeg[fperm[valid]]
    for c in range(ncores):
        blk = np.arange(c * NPC, (c + 1) * NPC)
        row_order = blk[np.argsort(prio[blk], kind="stable")]
        pos_order = blk[np.argsort(od_pos[blk], kind="stable")]
        growOf[pos_order] = row_order
    grow_node = np.full(n_nodes, -1, np.int64)  # node -> table row
    grow_node[fperm[valid]] = growOf[valid]
    rowOf = np.empty(NTOT, np.int64)  # table row -> dst position
    rowOf[growOf] = np.arange(NTOT)

    srcv = grow_node[src]
    cat = np.ones(E, np.int8)
    cat[srcv >= MOFF] = 2
    cat[srcv >= BLO] = 3
    cat[srcv >= WIN] = 4
    cat[srcv >= MOFF + WIN] = 5
    ncat = np.zeros((6, NTOT), np.int64)
    for cidx in range(1, 6):
        ncat[cidx] = np.bincount(dstf[cat == cidx], minlength=NTOT)
    n1, n2, n3, n4, n5 = ncat[1:6]
    deg3 = degf.reshape(ncores, NSLOT, TP)
    kf3 = kf.reshape(ncores, NSLOT, TP)
    # per band, minimize SA+SM+SB subject to per-node Hall constraints;
    # then realize a feasible per-node (toA2, toA3, toB3, toB4) greedily.
    SA_list, SM_list, SB_list, R_list = [], [], [], []
    toA2 = np.zeros(NTOT, np.int64)
    toB4 = np.zeros(NTOT, np.int64)
    toA3 = np.zeros(NTOT, np.int64)
    toB3 = np.zeros(NTOT, np.int64)
    rsh = lambda a: a.reshape(ncores, NSLOT, TP)
    for j in range(NSLOT):
        sl = (slice(None), j, slice(None))
        b1, b2, b3c, b4, b5 = (rsh(n1)[sl].ravel(), rsh(n2)[sl].ravel(),
                               rsh(n3)[sl].ravel(), rsh(n4)[sl].ravel(),
                               rsh(n5)[sl].ravel())
        dg = deg3[sl].ravel()
        m1, m5 = int(b1.max()), int(b5.max())
        m12, m45 = int((b1 + b2).max()), int((b4 + b5).max())
        m15, mdg = int((b1 + b5).max()), int(dg.max())
        best = (1 << 30, 0, 0, 0)
        for sa in range(m1, m1 + 33):
            for sm in range(0, 33):
                if sa + sm < m12:
                    continue
                sb = max(m5, m45 - sm, m15 - sa, mdg - sa - sm)
                if sa + sm + sb < best[0]:
                    best = (sa + sm + sb, sa, sm, sb)
        _, sa, sm, sb = best
        if sa + sm + sb < 8:
            sa = 8 - sm - sb
        SA_list.append(sa)
        SM_list.append(sm)
        SB_list.append(sb)
        R_list.append(max(1, (int(kf3[sl].max()) + 7) // 8))
        # greedy realization per node
        a_rem = sa - b1
        b_rem = sb - b5
        t2 = np.minimum(b2, a_rem); a_rem = a_rem - t2
        t4 = np.minimum(b4, b_rem); b_rem = b_rem - t4
        t3a = np.minimum(b3c, a_rem)
        t3b = np.minimum(b3c - t3a, b_rem)
        mu = (b2 - t2) + (b4 - t4) + (b3c - t3a - t3b)
        assert (mu <= sm).all() and (a_rem >= t3a).all()
        vids = (np.arange(ncores)[:, None] * (NSLOT * TP)
                + j * TP + np.arange(TP)[None, :]).ravel()
        toA2[vids], toB4[vids], toA3[vids], toB3[vids] = t2, t4, t3a, t3b
    S_list = [a + m + b for a, m, b in zip(SA_list, SM_list, SB_list)]
    # window of each edge: rank within (dst, cat) decides
    o1 = np.lexsort((np.arange(E), cat, dstf))
    # rank within each (dst, cat) run
    keys = dstf[o1] * 8 + cat[o1]
    starts = np.concatenate([[True], keys[1:] != keys[:-1]])
    run_id = np.cumsum(starts) - 1
    run_start = np.nonzero(starts)[0]
    rank = np.arange(E) - run_start[run_id]
    win = np.zeros(E, np.int8)  # 0=A 1=M 2=B
    co, ct, rk = cat[o1], dstf[o1], rank
    wo = np.zeros(E, np.int8)
    wo[co == 1] = 0
    wo[co == 5] = 2
    c2 = co == 2
    wo[c2] = np.where(rk[c2] < toA2[ct[c2]], 0, 1)
    c4 = co == 4
    wo[c4] = np.where(rk[c4] < toB4[ct[c4]], 2, 1)
    c3 = co == 3
    wo[c3] = np.where(
        rk[c3] < toA3[ct[c3]], 0,
        np.where(rk[c3] < toA3[ct[c3]] + toB3[ct[c3]], 2, 1))
    win[o1] = wo
    order = np.lexsort((np.arange(E), win, dstf))
    src_s = srcv[order]
    dst_s = dstf[order]
    win_s = win[order]
    segc = np.bincount(dst_s, minlength=NTOT)
    offs_seg = np.zeros(NTOT + 1, np.int64)
    np.cumsum(segc, out=offs_seg[1:])
    pos_in_seg = np.arange(E) - offs_seg[dst_s]
    cA = np.bincount(dst_s[win_s == 0], minlength=NTOT)
    cM = np.bincount(dst_s[win_s == 1], minlength=NTOT)
    offs = np.concatenate([[0], np.cumsum(S_list)]).astype(np.int64)
    F_tot = int(offs[-1])

    SW = max(S_list)
    sa_of = np.repeat(np.tile(np.array(SA_list), ncores), TP)
    sm_of = np.repeat(np.tile(np.array(SM_list), ncores), TP)
    part_base = np.where(
        win_s == 0, 0,
        np.where(win_s == 1, sa_of[dst_s], sa_of[dst_s] + sm_of[dst_s]))
    part_prev = np.where(
        win_s == 0, 0,
        np.where(win_s == 1, cA[dst_s], cA[dst_s] + cM[dst_s]))
    slot = pos_in_seg - part_prev + part_base
    ell_t = np.zeros((NTOT, SW), np.int32)
    ell_t[dst_s, slot] = src_s.astype(np.int32)
    real = np.zeros((NTOT, SW), bool)
    real[dst_s, slot] = True

    baseB_t = NTOT - WIN

    feat = np.ascontiguousarray(feat, dtype=np.float32)
    feat_pad = np.zeros((NTOT, D), np.float32)
    feat_pad[valid] = feat[fperm[valid]]
    feat_b16 = feat_pad.astype(ml_dtypes.bfloat16)  # [NTOT, D] 2B
    feat_b16_f32 = np.ascontiguousarray(feat_b16).view(np.float32)  # [NTOT, 64]

    groups = []
    g0, curW = 0, 0
    for j in range(NSLOT):
        if j > g0 and (curW + S_list[j] > SBUDGET or j - g0 >= GMAX):
            groups.append((g0, j))
            g0, curW = j, 0
        curW += S_list[j]
    groups.append((g0, NSLOT))

    in_maps = []
    for c in range(ncores):
        vids = np.arange(c * NPC, (c + 1) * NPC)
        bigp = np.full((TP, F_tot), NEG_BIG, np.float32)
        wtA_parts, wtM_parts, wtB_parts = [], [], []
        for j in range(NSLOT):
            vj = vids[j * TP : (j + 1) * TP]
            sa, sm, sb = SA_list[j], SM_list[j], SB_list[j]
            S_j = S_list[j]
            et = ell_t[vj, :S_j]
            rl = real[vj, :S_j]
            wtA_parts.append(np.where(rl[:, :sa], et[:, :sa], 0).astype(np.int32))
            wtM_parts.append(np.where(
                rl[:, sa : sa + sm], et[:, sa : sa + sm] - MOFF, 0
            ).astype(np.int32))
            wtB_parts.append(np.where(
                rl[:, sa + sm :], et[:, sa + sm :] - baseB_t, 0
            ).astype(np.int32))
            bigp[:, offs[j] : offs[j] + S_j] = np.where(rl, 0.0, NEG_BIG)
        wtA, wtM, wtB = [], [], []
        for (g0, g1) in groups:
            wtA.append(_wrap_idx(np.concatenate(wtA_parts[g0:g1], axis=1)))
            wtM.append(_wrap_idx(np.concatenate(wtM_parts[g0:g1], axis=1)))
            wtB.append(_wrap_idx(np.concatenate(wtB_parts[g0:g1], axis=1)))
        srctA_w = np.concatenate(wtA, axis=1)
        srctM_w = np.concatenate(wtM, axis=1)
        srctB_w = np.concatenate(wtB, axis=1)
        if srctA_w.shape[1] == 0:
            srctA_w = np.zeros((TP, 8), np.int16)
        if srctM_w.shape[1] == 0:
            srctM_w = np.zeros((TP, 8), np.int16)
        if srctB_w.shape[1] == 0:
            srctB_w = np.zeros((TP, 8), np.int16)

        kfc = kf[vids].reshape(NSLOT, TP).T.astype(np.float32)
        degc = degf[vids].reshape(NSLOT, TP).T
        am = np.where(degc > 0, 1.0 / np.maximum(kfc, 1.0), 0.0).astype(np.float32)
        # c1: deg-0 nodes keep their own feature twice (h_homo = feat + hr
        # with hr = feat); acc init = c1 * own-feat via ACT scale
        c1 = np.where(degc > 0, 1.0, 2.0).astype(np.float32)
        # G table (and featT, used to fill it) are in ROW order
        rows_pos = rowOf[vids]  # table row -> dst position, this core
        featT = feat_pad[rows_pos].T.copy()
        g_loc = np.zeros((NPC, GW), np.float32)
        g_loc[:, FOFF:] = feat_b16_f32[rows_pos]
        # own-rows gather indices (core-local): dst pos -> local row
        srctO_w = _wrap_idx(
            (growOf[vids] - c * NPC).reshape(NSLOT, TP).T.astype(np.int32))
        in_maps.append(
            {
                "g_loc": g_loc,
                "featT": featT,
                "bigp": bigp,
                "srctA": srctA_w,
                "srctM": srctM_w,
                "srctB": srctB_w,
                "srctO": srctO_w,
                "kf": kfc,
                "am": am,
                "c1": c1,
            }
        )

    sched = {
        "NTOT": NTOT,
        "NPC": NPC,
        "NSLOT": NSLOT,
        "SA": SA_list,
        "SM": SM_list,
        "SB": SB_list,
        "S": S_list,
        "R": R_list,
        "offs": offs.tolist(),
        "F_tot": F_tot,
        "groups": groups,
        "baseB_t": baseB_t,
        "MOFF": MOFF,
    }
    return sched, in_maps, fperm, valid


# ----------------------------------------------------------------------------
# Bass program builder (SPMD: one program; per-core variation is data only)
# ----------------------------------------------------------------------------
def _build_bass(sched, ncores, debug=False):
    STAGE = int(os.environ.get("K_STAGE", "99"))
    REPS = int(os.environ.get("K_REPS", "1"))
    PESUB = os.environ.get("K_PESUB", "0") == "1"
    NQ = int(os.environ.get("K_QUEUES", "4"))
    import concourse.bass as bass
    import concourse.bacc as bacc
    import concourse.tile as tile
    from concourse import mybir
    from concourse.masks import make_identity

    f32 = mybir.dt.float32
    bf16 = mybir.dt.bfloat16
    i16 = mybir.dt.int16
    NTOT, NPC, NSLOT = sched["NTOT"], sched["NPC"], sched["NSLOT"]
    SA_list, SM_list, SB_list, S_list, R_list = (
        sched["SA"], sched["SM"], sched["SB"], sched["S"], sched["R"],
    )
    offs, F_tot, groups = sched["offs"], sched["F_tot"], sched["groups"]
    baseB_t = sched["baseB_t"]
    MOFF = sched["MOFF"]
    CW = max(R_list) * 8
    SW = max(S_list)

    nc = bacc.Bacc(None, num_devices=ncores, dynamic_dma_scratch_size=SCRATCH,
                   num_swdge_queues=NQ)

    g_loc = nc.dram_tensor("g_loc", [NPC, GW], f32, kind="ExternalInput")
    featT = nc.dram_tensor("featT", [D, NPC], f32, kind="ExternalInput")
    bigp = nc.dram_tensor("bigp", [TP, F_tot], f32, kind="ExternalInput")
    srctA = nc.dram_tensor("srctA", [TP, max(8 * sum(SA_list), 8)], i16,
                           kind="ExternalInput")
    srctM = nc.dram_tensor("srctM", [TP, max(8 * sum(SM_list), 8)], i16,
                           kind="ExternalInput")
    srctB = nc.dram_tensor("srctB", [TP, max(8 * sum(SB_list), 8)], i16,
                           kind="ExternalInput")
    srctO = nc.dram_tensor("srctO", [TP, 8 * NSLOT], i16, kind="ExternalInput")
    kf_d = nc.dram_tensor("kf", [TP, NSLOT], f32, kind="ExternalInput")
    am_d = nc.dram_tensor("am", [TP, NSLOT], f32, kind="ExternalInput")
    c1_d = nc.dram_tensor("c1", [TP, NSLOT], f32, kind="ExternalInput")
    wmlp = nc.dram_tensor("wmlp", [D, C], f32, kind="ExternalInput")
    bmlp = nc.dram_tensor("bmlp", [C, 1], f32, kind="ExternalInput")
    wlin = nc.dram_tensor("wlin", [D, D], f32, kind="ExternalInput")
    blin = nc.dram_tensor("blin", [D, 1], f32, kind="ExternalInput")
    outT = nc.dram_tensor("outT", [D, NPC], f32, kind="ExternalOutput")

    gW = []
    for (g0, g1) in groups:
        gW.append((sum(SA_list[g0:g1]), sum(SM_list[g0:g1]),
                   sum(SB_list[g0:g1])))
    offsA = np.concatenate([[0], np.cumsum([a for a, _, _ in gW])]).astype(int)
    offsM = np.concatenate([[0], np.cumsum([m for _, m, _ in gW])]).astype(int)
    offsB = np.concatenate([[0], np.cumsum([b for _, _, b in gW])]).astype(int)

    with tile.TileContext(nc) as tc:
        with (
            tc.tile_pool(name="persist", bufs=1) as pp,
            tc.tile_pool(name="dram", bufs=1, space="DRAM") as dp,
        ):
            g_stage = dp.tile([NPC, GW], f32)

            # K_REPS: repeat the whole computation in-program for
            # dispatch-floor-free HW timing (T(N) ~ floor + N*hw).
            for _rep in range(REPS):
                g_aug = dp.tile(
                    [NTOT, GW], f32, name=f"g_aug{_rep}",
                    addr_space="Shared" if os.environ.get("K_SHARED", "1") == "1"
                    else "Local")
                # feat-bf16 cols (host-staged) -> Internal staging
                # rows; t columns are written by phase A below.
                nc.sync.dma_start(g_stage[:, FOFF:], g_loc[:, FOFF:])
                ident = pp.tile([TP, TP], f32)
                make_identity(nc, ident[:])
                identb = pp.tile([TP, TP], bf16)
                nc.vector.tensor_copy(out=identb[:], in_=ident[:])
                wmlp_t = pp.tile([D, C], f32)
                nc.sync.dma_start(wmlp_t[:], wmlp[:])
                bmlp_t = pp.tile([C, 1], f32)
                nc.sync.dma_start(bmlp_t[:], bmlp[:])
                wlin_t = pp.tile([D, D], f32)
                nc.sync.dma_start(wlin_t[:], wlin[:])
                wlin_b = pp.tile([D, D], bf16)
                nc.vector.tensor_copy(out=wlin_b[:], in_=wlin_t[:])
                blin_t = pp.tile([D, 1], f32)
                nc.sync.dma_start(blin_t[:], blin[:])
                kf_t = pp.tile([TP, NSLOT], f32)
                nc.sync.dma_start(kf_t[:], kf_d[:])
                am_t = pp.tile([TP, NSLOT], f32)
                nc.sync.dma_start(am_t[:], am_d[:])
                c1_t = pp.tile([TP, NSLOT], f32)
                nc.sync.dma_start(c1_t[:], c1_d[:])

                km1 = pp.tile([TP, NSLOT], f32)
                nc.vector.tensor_scalar(
                    out=km1[:], in0=kf_t[:], scalar1=-1.0, scalar2=None,
                    op0=mybir.AluOpType.add,
                )
                iota_i = pp.tile([TP, CW], mybir.dt.int32)
                nc.gpsimd.iota(out=iota_i[:], pattern=[[1, CW]], base=0,
                               channel_multiplier=0)
                iota_f = pp.tile([TP, CW], f32)
                nc.vector.tensor_copy(out=iota_f[:], in_=iota_i[:])
                zrow = pp.tile([TP, max(SW, 24)], f32)
                nc.vector.memset(zrow[:], 0.0)
                tau_all = pp.tile([TP, NSLOT], f32)
                t_own = pp.tile([TP, NSLOT * C], f32)

                # ---------------- Phase A: t = tanh(feat @ W_mlp + b) ----------
                with (
                    tc.tile_pool(name="pa", bufs=3) as pa,
                    tc.tile_pool(name="psA", bufs=2, space="PSUM") as psA,
                ):
                    o = 0
                    while o < NPC:
                        w = min(512, NPC - o)
                        ft = pa.tile([D, w], f32, tag="ft")
                        nc.sync.dma_start(ft[:], featT[:, o : o + w])
                        pm = psA.tile([C, w], f32, tag="pm")
                        nc.tensor.matmul(out=pm[:], lhsT=wmlp_t[:], rhs=ft[:],
                                         start=True, stop=True)
                        th = pa.tile([C, w], f32, tag="th")
                        nc.scalar.activation(out=th[:], in_=pm[:],
                                             func=mybir.ActivationFunctionType.Tanh,
                                             bias=bmlp_t[:])
                        for i in range(w // TP):
                            j = (o + i * TP) // TP
                            pt = psA.tile([TP, C], f32, tag="pt")
                            nc.tensor.transpose(out=pt[:], in_=th[:, i * TP : (i + 1) * TP],
                                                identity=ident[:C, :C])
                            nc.scalar.activation(
                                out=t_own[:, j * C : (j + 1) * C], in_=pt[:],
                                func=mybir.ActivationFunctionType.Copy)
                            nc.sync.dma_start(
                                g_stage[o + i * TP : o + (i + 1) * TP, :C],
                                t_own[:, j * C : (j + 1) * C],
                            )
                        o += w

                if STAGE >= 1:
                    nc.gpsimd.collective_compute(
                        "AllGather",
                        mybir.AluOpType.bypass,
                        replica_groups=[list(range(ncores))],
                        ins=[g_stage[:].opt()],
                        outs=[g_aug[:].opt()],
                    )

                # own-rows gather (core-local; overlaps the AllGather):
                # t_own + own-feat for every dst tile, in dst order
                t_all = pp.tile([TP, NSLOT * GW], f32)
                t_all3 = t_all[:].rearrange("p (w c) -> p w c", c=GW)
                bvO = t_all[:].bitcast(bf16)  # [TP, NSLOT*2*GW]
                wiO = pp.tile([TP, 8 * NSLOT], mybir.dt.int16)
                nc.sync.dma_start(wiO[:], srctO[:])
                qrr0 = 0
                for c0 in range(0, NSLOT, GCH):
                    cw = min(GCH, NSLOT - c0)
                    nc.gpsimd.dma_gather(
                        out_ap=t_all3[:, c0 : c0 + cw, :],
                        in_ap=g_stage[:, :],
                        idxs_ap=wiO[:, 8 * c0 : 8 * (c0 + cw)],
                        num_idxs=cw * TP, num_idxs_reg=cw * TP,
                        elem_size=GW,
                        queue_num=(qrr0 := qrr0 + 1) % NQ,
                    )

                if STAGE < 5:
                    for j in range(NSLOT):
                        nc.sync.dma_start(outT[:, j * TP : (j + 1) * TP], ident[:])
                # ---------------- Phase B ---------------------------------------
                with (
                    tc.tile_pool(name="pb", bufs=2) as pb,
                    tc.tile_pool(name="pmsg", bufs=2) as pmsg,
                    tc.tile_pool(name="pms", bufs=8) as pms,
                    tc.tile_pool(name="psB", bufs=2, space="PSUM") as psB,
                ):
                    qrr = 0
                    for gi, (j0, j1) in enumerate(groups):
                        if STAGE <= 1:
                            break
                        nt = j1 - j0
                        W = offs[j1] - offs[j0]
                        nA, nM, nB = gW[gi]
                        big = pb.tile([TP, W], f32, tag="big")
                        nc.sync.dma_start(big[:], bigp[:, offs[j0] : offs[j1]])

                        wiA = pb.tile([TP, 8 * max(nA, 1)], i16, tag="wiA")
                        wiM = pb.tile([TP, 8 * max(nM, 1)], i16, tag="wiM")
                        wiB = pb.tile([TP, 8 * max(nB, 1)], i16, tag="wiB")
                        if nA:
                            nc.sync.dma_start(
                                wiA[:, : 8 * nA],
                                srctA[:, 8 * offsA[gi] : 8 * offsA[gi + 1]])
                        if nM:
                            nc.sync.dma_start(
                                wiM[:, : 8 * nM],
                                srctM[:, 8 * offsM[gi] : 8 * offsM[gi + 1]])
                        if nB:
                            nc.sync.dma_start(
                                wiB[:, : 8 * nB],
                                srctB[:, 8 * offsB[gi] : 8 * offsB[gi + 1]])

                        WA_t = min(WIN, NTOT)
                        tsrc = pb.tile([TP, W * GW], f32, tag="tsrc")
                        tsrc3 = tsrc[:].rearrange("p (w c) -> p w c", c=GW)
                        for c0 in range(0, nA, GCH):
                            cw = min(GCH, nA - c0)
                            nc.gpsimd.dma_gather(
                                out_ap=tsrc3[:, c0 : c0 + cw, :],
                                in_ap=g_aug[:WA_t, :],
                                idxs_ap=wiA[:, 8 * c0 : 8 * (c0 + cw)],
                                num_idxs=cw * TP, num_idxs_reg=cw * TP,
                                elem_size=GW,
                                queue_num=(qrr := qrr + 1) % NQ,
                            )
                        for c0 in range(0, nM, GCH):
                            cw = min(GCH, nM - c0)
                            nc.gpsimd.dma_gather(
                                out_ap=tsrc3[:, nA + c0 : nA + c0 + cw, :],
                                in_ap=g_aug[MOFF : MOFF + WIN, :],
                                idxs_ap=wiM[:, 8 * c0 : 8 * (c0 + cw)],
                                num_idxs=cw * TP, num_idxs_reg=cw * TP,
                                elem_size=GW,
                                queue_num=(qrr := qrr + 1) % NQ,
                            )
                        for c0 in range(0, nB, GCH):
                            cw = min(GCH, nB - c0)
                            nc.gpsimd.dma_gather(
                                out_ap=tsrc3[:, nA + nM + c0 : nA + nM + c0 + cw, :],
                                in_ap=g_aug[baseB_t : baseB_t + WIN, :],
                                idxs_ap=wiB[:, 8 * c0 : 8 * (c0 + cw)],
                                num_idxs=cw * TP, num_idxs_reg=cw * TP,
                                elem_size=GW,
                                queue_num=(qrr := qrr + 1) % NQ,
                            )
                        if STAGE <= 2:
                            continue

                        # subtract own-node t (broadcast over slots) per tile/part
                        ndg = pb.tile([TP, W], f32, tag="ndg")
                        for tci in range(nt):
                            j = j0 + tci
                            tdst_b = t_all3[:, j, :C]
                            cA0 = sum(SA_list[j0:j])
                            cM0 = nA + sum(SM_list[j0:j])
                            cB0 = nA + nM + sum(SB_list[j0:j])
                            for (c0, ns) in ((cA0, SA_list[j]),
                                             (cM0, SM_list[j]),
                                             (cB0, SB_list[j])):
                                if ns == 0:
                                    continue
                                sl = tsrc[:, c0 * GW : (c0 + ns) * GW].rearrange(
                                    "p (s c) -> p s c", s=ns)[:, :, :C]
                                nc.vector.tensor_tensor(
                                    out=sl, in0=sl,
                                    in1=tdst_b.unsqueeze(1).to_broadcast(
                                        [TP, ns, C]),
                                    op=mybir.AluOpType.subtract,
                                )
                        nc.vector.tensor_reduce(
                            out=ndg[:],
                            in_=tsrc3[:, :, :C],
                            axis=mybir.AxisListType.X, op=mybir.AluOpType.add,
                            apply_absolute_value=True, negate=True,
                        )
                        # per-tile contiguous nd (A|B parts adjacent) + bigpad
                        nd = pb.tile([TP, W], f32, tag="nd")
                        for tci in range(nt):
                            j = j0 + tci
                            sa, sm, sb = SA_list[j], SM_list[j], SB_list[j]
                            a0 = offs[j] - offs[j0]
                            cA0 = sum(SA_list[j0:j])
                            cM0 = nA + sum(SM_list[j0:j])
                            cB0 = nA + nM + sum(SB_list[j0:j])
                            for (c0, ns, u0) in ((cA0, sa, 0), (cM0, sm, sa),
                                                 (cB0, sb, sa + sm)):
                                if ns:
                                    nc.vector.tensor_copy(
                                        out=nd[:, a0 + u0 : a0 + u0 + ns],
                                        in_=ndg[:, c0 : c0 + ns])
                        nc.vector.tensor_tensor(out=nd[:], in0=nd[:], in1=big[:],
                                                op=mybir.AluOpType.add)

                        # selection per tile
                        ndw = pb.tile([TP, W], f32, tag="ndw")
                        nc.vector.tensor_copy(out=ndw[:], in_=nd[:])
                        cand = pb.tile([TP, CW], f32, tag="cand")
                        eqs = pb.tile([TP, CW], f32, tag="eqs")
                        maskf = pb.tile([TP, W], f32, tag="maskf")
                        eqm = pb.tile([TP, W], f32, tag="eqm")
                        prf = pb.tile([TP, W], f32, tag="prf")
                        cnt1 = pb.tile([TP, 1], f32, tag="cnt1")
                        for tci in range(nt):
                            j = j0 + tci
                            S_j = S_list[j]
                            R_j = R_list[j]
                            a0 = offs[j] - offs[j0]
                            ndw_j = ndw[:, a0 : a0 + S_j]
                            for r in range(R_j):
                                nc.vector.max(out=cand[:, r * 8 : (r + 1) * 8],
                                              in_=ndw_j)
                                if r < R_j - 1:
                                    nc.vector.match_replace(
                                        out=ndw_j,
                                        in_to_replace=cand[:, r * 8 : (r + 1) * 8],
                                        in_values=ndw_j, imm_value=NEG_BIG)
                            wcw = R_j * 8
                            nc.vector.tensor_scalar(
                                out=eqs[:, :wcw], in0=iota_f[:, :wcw],
                                scalar1=km1[:, j : j + 1], scalar2=None,
                                op0=mybir.AluOpType.is_equal)
                            nc.vector.tensor_tensor(
                                out=eqs[:, :wcw], in0=eqs[:, :wcw],
                                in1=cand[:, :wcw], op=mybir.AluOpType.mult)
                            nc.vector.tensor_reduce(
                                out=tau_all[:, j : j + 1], in_=eqs[:, :wcw],
                                axis=mybir.AxisListType.X, op=mybir.AluOpType.add)
                            nd_j = nd[:, a0 : a0 + S_j]
                            ltm_j = maskf[:, a0 : a0 + S_j]
                            eqm_j = eqm[:, a0 : a0 + S_j]
                            prf_j = prf[:, a0 : a0 + S_j]
                            nc.vector.tensor_scalar(
                                out=ltm_j, in0=nd_j, scalar1=tau_all[:, j : j + 1],
                                scalar2=None, op0=mybir.AluOpType.is_gt)
                            nc.vector.tensor_scalar(
                                out=eqm_j, in0=nd_j, scalar1=tau_all[:, j : j + 1],
                                scalar2=None, op0=mybir.AluOpType.is_equal)
                            nc.vector.tensor_reduce(
                                out=cnt1[:], in_=ltm_j,
                                axis=mybir.AxisListType.X, op=mybir.AluOpType.add)
                            nc.vector.tensor_tensor(
                                out=cnt1[:], in0=kf_t[:, j : j + 1], in1=cnt1[:],
                                op=mybir.AluOpType.subtract)
                            nc.vector.tensor_tensor_scan(
                                out=prf_j, data0=eqm_j, data1=zrow[:, :S_j],
                                initial=0.0, op0=mybir.AluOpType.add,
                                op1=mybir.AluOpType.add)
                            nc.vector.tensor_scalar(
                                out=prf_j, in0=prf_j, scalar1=cnt1[:], scalar2=None,
                                op0=mybir.AluOpType.is_le)
                            nc.vector.tensor_tensor(
                                out=eqm_j, in0=eqm_j, in1=prf_j,
                                op=mybir.AluOpType.mult)
                            nc.vector.tensor_tensor(
                                out=ltm_j, in0=ltm_j, in1=eqm_j,
                                op=mybir.AluOpType.add)

                        if STAGE <= 3:
                            continue
                        # mask (x 1/k) back to gather layout, as bf16
                        maskg = pb.tile([TP, W], f32, tag="maskg")
                        for tci in range(nt):
                            j = j0 + tci
                            sa, sm, sb = SA_list[j], SM_list[j], SB_list[j]
                            a0 = offs[j] - offs[j0]
                            cA0 = sum(SA_list[j0:j])
                            cM0 = nA + sum(SM_list[j0:j])
                            cB0 = nA + nM + sum(SB_list[j0:j])
                            for (c0, ns, u0) in ((cA0, sa, 0), (cM0, sm, sa),
                                                 (cB0, sb, sa + sm)):
                                if ns:
                                    nc.vector.tensor_scalar(
                                        out=maskg[:, c0 : c0 + ns],
                                        in0=maskf[:, a0 + u0 : a0 + u0 + ns],
                                        scalar1=am_t[:, j : j + 1], scalar2=None,
                                        op0=mybir.AluOpType.mult)
                        bv = tsrc[:].bitcast(bf16)  # [TP, W*256]

                        if STAGE <= 4:
                            continue
                        # per tile: ACT scales each slot's gathered feat by its
                        # mask (per-partition scalar), PE accumulates the
                        # transposed messages straight into group PSUM via
                        # identity-rhs matmuls (start at the c1-scaled own
                        # feature, stop at the last slot):
                        #   psTg[:, tci*D:][d, p] = sum_s m_s[p] * F_s[p, d]
                        psTg = psB.tile([TP, nt * D], f32, tag="psTg")
                        for tci in range(nt):
                            j = j0 + tci
                            sa, sm, sb = SA_list[j], SM_list[j], SB_list[j]
                            S_j = S_list[j]
                            cA0 = sum(SA_list[j0:j])
                            cM0 = nA + sum(SM_list[j0:j])
                            cB0 = nA + nM + sum(SB_list[j0:j])
                            pout = psTg[:, tci * D : (tci + 1) * D]
                            msga = pms.tile([TP, D], bf16, tag="msga")
                            nc.scalar.activation(
                                out=msga[:],
                                in_=bvO[:, j * 2 * GW + D : j * 2 * GW + 2 * D],
                                func=mybir.ActivationFunctionType.Copy,
                                scale=c1_t[:, j : j + 1])
                            nc.tensor.matmul(out=pout, lhsT=msga[:],
                                             rhs=identb[:], start=True,
                                             stop=(S_j == 0))
                            for u in range(S_j):
                                col = (cA0 + u if u < sa else
                                       cM0 + (u - sa) if u < sa + sm else
                                       cB0 + (u - sa - sm))
                                msga = pms.tile([TP, D], bf16, tag="msga")
                                nc.scalar.activation(
                                    out=msga[:],
                                    in_=bv[:, col * 2 * GW + D
                                           : col * 2 * GW + 2 * D],
                                    func=mybir.ActivationFunctionType.Copy,
                                    scale=maskg[:, col : col + 1])
                                nc.tensor.matmul(out=pout, lhsT=msga[:],
                                                 rhs=identb[:], start=False,
                                                 stop=(u == S_j - 1))
                        hTg = pmsg.tile([TP, nt * D], bf16, tag="hTg")
                        nc.scalar.activation(
                            out=hTg[:], in_=psTg[:],
                            func=mybir.ActivationFunctionType.Copy)
                        opg = psB.tile([D, nt * TP], f32, tag="opg")
                        for c0 in range(0, nt * TP, 512):
                            cw = min(512, nt * TP - c0)
                            nc.tensor.matmul(
                                out=opg[:, c0 : c0 + cw], lhsT=wlin_b[:],
                                rhs=hTg[:, c0 : c0 + cw], start=True, stop=True)
                        otg = pmsg.tile([D, nt * TP], f32, tag="otg")
                        nc.scalar.activation(
                            out=otg[:], in_=opg[:],
                            func=mybir.ActivationFunctionType.Identity,
                            bias=blin_t[:])
                        nc.sync.dma_start(
                            outT[:, j0 * TP : j1 * TP], otg[:])
    nc.finalize()
    return nc


# ----------------------------------------------------------------------------
# Runner
# ----------------------------------------------------------------------------
_CACHE = {}


def _get_program(sched, ncores):
    key = (tuple(sched["S"]), tuple(sched["SA"]), tuple(sched["R"]), sched["NTOT"])
    if key not in _CACHE:
        _CACHE[key] = _build_bass(sched, ncores)
    return _CACHE[key]


def kernel(**inputs):
    feat = np.asarray(inputs["feat"], dtype=np.float32)
    src = np.asarray(inputs["src"]).astype(np.int64)
    dst = np.asarray(inputs["dst"]).astype(np.int64)
    W_mlp = np.asarray(inputs["W_mlp"], dtype=np.float32)
    b_mlp = np.asarray(inputs["b_mlp"], dtype=np.float32)
    W_lin = np.asarray(inputs["W_lin"], dtype=np.float32)
    b_lin = np.asarray(inputs["b_lin"], dtype=np.float32)
    n = feat.shape[0]

    sched, in_maps, fperm, valid = _preprocess(src, dst, feat, n, NCORES)
    for m in in_maps:
        m["wmlp"] = np.ascontiguousarray(W_mlp)
        m["bmlp"] = np.ascontiguousarray(b_mlp.reshape(C, 1))
        m["wlin"] = np.ascontiguousarray(W_lin)
        m["blin"] = np.ascontiguousarray(b_lin.reshape(D, 1))

    nc = _get_program(sched, NCORES)

    from concourse.bass_utils import run_bass_kernel_spmd

    res = run_bass_kernel_spmd(nc, in_maps, list(range(NCORES)))

    full = np.concatenate([res.results[c]["outT"] for c in range(NCORES)], axis=1)
    out = np.empty((n, D), np.float32)
    out[fperm[valid]] = full.T[valid]
    return out

